# revision 1
# baseline (speedup 1.0000x reference)
"""FP8ScaledLayer kernel for Trainium2 (8 NeuronCores, SPMD data-parallel).

Computes out = x @ (weight * scale[:, None]).T + bias with
  x: [4, 4096, 4096] fp32, weight: [4096, 4096] fp16,
  scale_weight: [4096] fp32, bias: [4096] fp32  ->  out [4, 4096, 4096] fp32.

Sharding: data-parallel over tokens (B*S = 16384 -> 2048 rows/core).
Weight is small (33.5 MB fp16) and replicated; x is large (268 MB) and
sharded, which keeps every core compute-bound instead of DMA-bound.

Per-core kernel:
  - cast x fp32 -> fp16 with a DRAM->DRAM SWDGE cast-DMA,
  - DMA-transpose (XBAR) x16 and weight into K-major SBUF tiles,
  - 16x8x32 matmuls of [128k,128m]^T @ [128k,512n] accumulating in PSUM,
  - scale/bias applied to the fp32 PSUM result on VectorE
    (weight stays exact fp16; only x is quantized to fp16).
"""

import sys

if "/opt/trn_rl_repo" not in sys.path:
    sys.path.insert(0, "/opt/trn_rl_repo")

import numpy as np

import concourse.bass as bass
import concourse.mybir as mybir
import concourse.tile as tile
from concourse import bacc

P = 128
N_CORES = 8
B, S, K, N = 4, 4096, 4096, 4096
M_TOTAL = B * S
M_SH = M_TOTAL // N_CORES  # 2048 rows per core
KO = K // P  # 32
MO = M_SH // P  # 16
N_TILE = 512
NO = N // N_TILE  # 8

F32 = mybir.dt.float32
F16 = mybir.dt.float16
BF16 = mybir.dt.bfloat16

_CACHED_NC = None


def _build_nc():
    nc = bacc.Bacc(
        None,
        target_bir_lowering=False,
        num_swdge_queues=1,
        dynamic_dma_scratch_size=2048,
    )

    x = nc.dram_tensor("x", (M_SH, K), F32, kind="ExternalInput")
    w = nc.dram_tensor("weight", (N, K), F16, kind="ExternalInput")
    scale = nc.dram_tensor("scale_weight", (N,), F32, kind="ExternalInput")
    bias = nc.dram_tensor("bias", (N,), F32, kind="ExternalInput")
    out = nc.dram_tensor("out", (M_SH, N), F32, kind="ExternalOutput")

    with tile.TileContext(nc) as tc:
        with (
            tc.tile_pool(name="dram", bufs=1, space="DRAM") as dram,
            tc.tile_pool(name="xT", bufs=1) as xtp,
            tc.tile_pool(name="wT", bufs=2) as wtp,
            tc.tile_pool(name="sbrep", bufs=2) as sbp,
            tc.tile_pool(name="psum", bufs=4, space="PSUM") as pp,
            tc.tile_pool(name="osb", bufs=3) as op,
        ):
            def load_rep(pool_tile, src_handle, ncols):
                # partition-broadcast via HWDGE (stride-0 partition source)
                sl = src_handle[ncols]
                nc.scalar.dma_start(
                    out=pool_tile[:],
                    in_=bass.AP(tensor=sl.tensor, offset=sl.offset, ap=[[0, P], *sl.ap]),
                )

            # ---- x pipeline: SWDGE fp32->fp16 cast-DMA (DRAM->DRAM), then
            # XBAR-transpose into the SBUF-resident xT. Every DMA handoff
            # costs ~10-20us of completion-semaphore latency, so both stages
            # are deep-buffered through DRAM; only x loses precision (fp16
            # round-off), the weight stays exact fp16.
            x16 = dram.tile((M_SH, K), F16)
            xT = xtp.tile((P, MO, KO, P), F16)  # xT[p, mo, ko, m] = x[mo*128+m, ko*128+p]
            wts = {}
            sreps = {}

            # wT[p, ko, n] = w[no*512+n, ko*128+p]; passes 0 and 1 are
            # interleaved, so both weight tiles load upfront (2 chunks each
            # for an earlier first matmul).
            for no in (0, 1):
                wTn = wtp.tile((P, KO, N_TILE), F16, tag="wT")
                for j in range(2):
                    nc.sync.dma_start_transpose(
                        wTn[:, 16 * j:16 * (j + 1), :],
                        w[no * N_TILE:(no + 1) * N_TILE, 2048 * j:2048 * (j + 1)],
                    )
                wts[no] = wTn
                s_rep = sbp.tile((P, N_TILE), F32, tag="scale")
                b_rep = sbp.tile((P, N_TILE), F32, tag="bias")
                load_rep(s_rep, scale, slice(no * N_TILE, (no + 1) * N_TILE))
                load_rep(b_rep, bias, slice(no * N_TILE, (no + 1) * N_TILE))
                sreps[no] = (s_rep, b_rep)

            # x producer: one SWDGE fp32->fp16 cast-DMA per 128-row chunk
            # (DRAM->DRAM, deep-buffered), XBAR transpose consumes from DRAM.
            for mo in range(MO):
                rows = slice(mo * P, (mo + 1) * P)
                nc.gpsimd.dma_start(out=x16[rows, :], in_=x[rows, :])
                nc.sync.dma_start_transpose(xT[:, mo], x16[rows, :])

            # passes 0 and 1 interleaved per mo: halves the rate at which new
            # transposed-x chunks are consumed (13.8us/chunk), matching the
            # ~17-23us/chunk the serial SWDGE cast queue can produce
            for mo in range(MO):
                for no in (0, 1):
                    ncols = slice(no * N_TILE, (no + 1) * N_TILE)
                    wT = wts[no]
                    scale_rep, bias_rep = sreps[no]
                    ps = pp.tile((P, N_TILE), F32, tag="ps")
                    for ko in range(KO):
                        nc.tensor.matmul(
                            ps[:],
                            lhsT=xT[:, mo, ko, :],
                            rhs=wT[:, ko, :],
                            start=(ko == 0),
                            stop=(ko == KO - 1),
                        )
                    ot = op.tile((P, N_TILE), F32, tag="ot")
                    nc.vector.tensor_mul(ot[:], ps[:], scale_rep[:])
                    nc.vector.tensor_add(ot[:], ot[:], bias_rep[:])
                    eng = nc.scalar if mo % 2 == 0 else nc.sync
                    eng.dma_start(out[mo * P:(mo + 1) * P, ncols], ot[:])

            for no in range(2, NO):
                ncols = slice(no * N_TILE, (no + 1) * N_TILE)
                wT = wtp.tile((P, KO, N_TILE), F16, tag="wT")
                nc.sync.dma_start_transpose(wT, w[ncols, :])
                scale_rep = sbp.tile((P, N_TILE), F32, tag="scale")
                bias_rep = sbp.tile((P, N_TILE), F32, tag="bias")
                load_rep(scale_rep, scale, ncols)
                load_rep(bias_rep, bias, ncols)

                for mo in range(MO):
                    ps = pp.tile((P, N_TILE), F32, tag="ps")
                    for ko in range(KO):
                        nc.tensor.matmul(
                            ps[:],
                            lhsT=xT[:, mo, ko, :],
                            rhs=wT[:, ko, :],
                            start=(ko == 0),
                            stop=(ko == KO - 1),
                        )
                    ot = op.tile((P, N_TILE), F32, tag="ot")
                    nc.vector.tensor_mul(ot[:], ps[:], scale_rep[:])
                    nc.vector.tensor_add(ot[:], ot[:], bias_rep[:])
                    # alternate output writes across both HWDGE queues
                    eng = nc.scalar if mo % 2 == 0 else nc.sync
                    eng.dma_start(out[mo * P:(mo + 1) * P, ncols], ot[:])

    nc.finalize()
    return nc


def _get_nc():
    global _CACHED_NC
    if _CACHED_NC is None:
        _CACHED_NC = _build_nc()
    return _CACHED_NC


def _run(inputs, trace=False, **spmd_kwargs):
    from concourse.bass_utils import run_bass_kernel_spmd

    x = np.asarray(inputs["x"], dtype=np.float32).reshape(M_TOTAL, K)
    w = np.ascontiguousarray(np.asarray(inputs["weight"], dtype=np.float16))
    scale = np.ascontiguousarray(np.asarray(inputs["scale_weight"], dtype=np.float32))
    bias = np.ascontiguousarray(np.asarray(inputs["bias"], dtype=np.float32))

    in_maps = []
    for c in range(N_CORES):
        in_maps.append(
            {
                "x": np.ascontiguousarray(x[c * M_SH:(c + 1) * M_SH]),
                "weight": w,
                "scale_weight": scale,
                "bias": bias,
            }
        )

    nc = _get_nc()
    res = run_bass_kernel_spmd(
        nc, in_maps, core_ids=list(range(N_CORES)), trace=trace, **spmd_kwargs
    )
    out = np.concatenate([res.results[c]["out"] for c in range(N_CORES)], axis=0)
    return out.reshape(B, S, N), res


def kernel(x, weight, scale_weight, bias):
    out, _ = _run({"x": x, "weight": weight, "scale_weight": scale_weight, "bias": bias})
    return out



# revision 2
# speedup vs baseline: 1.0155x; 1.0155x over previous
"""FP8ScaledLayer kernel for Trainium2 (8 NeuronCores, SPMD data-parallel).

Computes out = x @ (weight * scale[:, None]).T + bias with
  x: [4, 4096, 4096] fp32, weight: [4096, 4096] fp16,
  scale_weight: [4096] fp32, bias: [4096] fp32  ->  out [4, 4096, 4096] fp32.

Sharding: data-parallel over tokens (B*S = 16384 -> 2048 rows/core).
Weight is small (33.5 MB fp16) and replicated; x is large (268 MB) and
sharded, which keeps every core compute-bound instead of DMA-bound.

Per-core kernel:
  - cast x fp32 -> fp16 with a DRAM->DRAM SWDGE cast-DMA,
  - DMA-transpose (XBAR) x16 and weight into K-major SBUF tiles,
  - 16x8x32 matmuls of [128k,128m]^T @ [128k,512n] accumulating in PSUM,
  - scale/bias applied to the fp32 PSUM result on VectorE
    (weight stays exact fp16; only x is quantized to fp16).

Queue roles (engine program order == HW queue order, sem-gated in order):
  - gpsimd: the 16 SWDGE cast-DMAs, nothing else.
  - sync:   producer transposes only: wT0 (4 pieces), xT0, wT1 (4 pieces),
            xT1..xT15, then per-pass [wT_no (2 pieces) + scale/bias reps].
  - scalar: scale/bias reps for no=0,1 upfront, then all 128 output writes
            in tile order (each gated only on its own epilogue).
The interleaved (no 0,1) phase ends staggered: mo 12..15 run no=0 tiles
then no=1 tiles, so the wT2 transpose has ~55us of matmul cover after
its pool slot frees instead of 14us.
"""

import sys

if "/opt/trn_rl_repo" not in sys.path:
    sys.path.insert(0, "/opt/trn_rl_repo")

import numpy as np

import concourse.bass as bass
import concourse.mybir as mybir
import concourse.tile as tile
from concourse import bacc

P = 128
N_CORES = 8
B, S, K, N = 4, 4096, 4096, 4096
M_TOTAL = B * S
M_SH = M_TOTAL // N_CORES  # 2048 rows per core
KO = K // P  # 32
MO = M_SH // P  # 16
N_TILE = 512
NO = N // N_TILE  # 8

F32 = mybir.dt.float32
F16 = mybir.dt.float16

_CACHED_NC = None


def _build_nc():
    nc = bacc.Bacc(
        None,
        target_bir_lowering=False,
        num_swdge_queues=1,
        dynamic_dma_scratch_size=2048,
    )

    x = nc.dram_tensor("x", (M_SH, K), F32, kind="ExternalInput")
    w = nc.dram_tensor("weight", (N, K), F16, kind="ExternalInput")
    scale = nc.dram_tensor("scale_weight", (N,), F32, kind="ExternalInput")
    bias = nc.dram_tensor("bias", (N,), F32, kind="ExternalInput")
    out = nc.dram_tensor("out", (M_SH, N), F32, kind="ExternalOutput")

    with tile.TileContext(nc) as tc:
        with (
            tc.tile_pool(name="dram", bufs=1, space="DRAM") as dram,
            tc.tile_pool(name="xT", bufs=1) as xtp,
            tc.tile_pool(name="wT", bufs=2) as wtp,
            tc.tile_pool(name="sbrep", bufs=2) as sbp,
            tc.tile_pool(name="psum", bufs=4, space="PSUM") as pp,
            tc.tile_pool(name="osb", bufs=3) as op,
        ):
            def load_rep(eng, pool_tile, src_handle, ncols):
                # partition-broadcast via HWDGE (stride-0 partition source)
                sl = src_handle[ncols]
                eng.dma_start(
                    out=pool_tile[:],
                    in_=bass.AP(tensor=sl.tensor, offset=sl.offset, ap=[[0, P], *sl.ap]),
                )

            x16 = dram.tile((M_SH, K), F16)
            xT = xtp.tile((P, MO, KO, P), F16)  # xT[p, mo, ko, m] = x[mo*128+m, ko*128+p]
            wts = {}
            sreps = {}

            def make_wt(no, pieces, eng):
                # wT[p, ko, n] = w[no*512+n, ko*128+p], transposed in `pieces`
                # column-chunks so the first tile's dependencies land early.
                wTn = wtp.tile((P, KO, N_TILE), F16, tag="wT")
                cw = K // pieces
                cko = KO // pieces
                for j in range(pieces):
                    eng.dma_start_transpose(
                        wTn[:, cko * j:cko * (j + 1), :],
                        w[no * N_TILE:(no + 1) * N_TILE, cw * j:cw * (j + 1)],
                    )
                wts[no] = wTn

            def make_reps(no, eng):
                s_rep = sbp.tile((P, N_TILE), F32, tag="scale")
                b_rep = sbp.tile((P, N_TILE), F32, tag="bias")
                ncols = slice(no * N_TILE, (no + 1) * N_TILE)
                load_rep(eng, s_rep, scale, ncols)
                load_rep(eng, b_rep, bias, ncols)
                sreps[no] = (s_rep, b_rep)

            def cast_chunk(mo):
                rows = slice(mo * P, (mo + 1) * P)
                nc.gpsimd.dma_start(out=x16[rows, :], in_=x[rows, :])

            def transpose_chunk(mo):
                rows = slice(mo * P, (mo + 1) * P)
                nc.sync.dma_start_transpose(xT[:, mo], x16[rows, :])

            def mm_tile(mo, no):
                ncols = slice(no * N_TILE, (no + 1) * N_TILE)
                wT = wts[no]
                scale_rep, bias_rep = sreps[no]
                ps = pp.tile((P, N_TILE), F32, tag="ps")
                for ko in range(KO):
                    nc.tensor.matmul(
                        ps[:],
                        lhsT=xT[:, mo, ko, :],
                        rhs=wT[:, ko, :],
                        start=(ko == 0),
                        stop=(ko == KO - 1),
                    )
                ot = op.tile((P, N_TILE), F32, tag="ot")
                nc.vector.tensor_mul(ot[:], ps[:], scale_rep[:])
                nc.vector.tensor_add(ot[:], ot[:], bias_rep[:])
                nc.scalar.dma_start(out[mo * P:(mo + 1) * P, ncols], ot[:])

            # ---- priming: first x chunk + first weight tile race to ready
            cast_chunk(0)                     # gpsimd
            make_reps(0, nc.scalar)           # scalar: tiny, before outputs
            make_reps(1, nc.scalar)
            make_wt(0, 4, nc.sync)            # sync: 4 x ~1MB pieces, no gate
            transpose_chunk(0)                # sync: gated on cast 0 only
            make_wt(1, 4, nc.sync)
            for mo in range(1, MO):
                cast_chunk(mo)
                transpose_chunk(mo)

            # ---- interleaved phase over no in {0,1}: halves the x-chunk
            # consumption rate (27.6us/chunk) to match the serial SWDGE cast
            # queue (~17-23us/chunk). Tail staggered: no=0 finishes 4 tiles
            # before no=1 so the wT2 transpose gets cover.
            STAG = 4
            for mo in range(MO - STAG):
                mm_tile(mo, 0)
                mm_tile(mo, 1)
            for mo in range(MO - STAG, MO):
                mm_tile(mo, 0)
            for mo in range(MO - STAG, MO):
                mm_tile(mo, 1)

            # ---- remaining passes: wT[no] transposes on sync ride behind the
            # x transposes (all done by ~350us) and are gated only on their
            # pool slot (freed a full pass earlier).
            for no in range(2, NO):
                make_wt(no, 2, nc.sync)
                make_reps(no, nc.sync)
                for mo in range(MO):
                    mm_tile(mo, no)

    nc.finalize()
    return nc


def _get_nc():
    global _CACHED_NC
    if _CACHED_NC is None:
        _CACHED_NC = _build_nc()
    return _CACHED_NC


def _run(inputs, trace=False, **spmd_kwargs):
    from concourse.bass_utils import run_bass_kernel_spmd

    x = np.asarray(inputs["x"], dtype=np.float32).reshape(M_TOTAL, K)
    w = np.ascontiguousarray(np.asarray(inputs["weight"], dtype=np.float16))
    scale = np.ascontiguousarray(np.asarray(inputs["scale_weight"], dtype=np.float32))
    bias = np.ascontiguousarray(np.asarray(inputs["bias"], dtype=np.float32))

    in_maps = []
    for c in range(N_CORES):
        in_maps.append(
            {
                "x": np.ascontiguousarray(x[c * M_SH:(c + 1) * M_SH]),
                "weight": w,
                "scale_weight": scale,
                "bias": bias,
            }
        )

    nc = _get_nc()
    res = run_bass_kernel_spmd(
        nc, in_maps, core_ids=list(range(N_CORES)), trace=trace, **spmd_kwargs
    )
    out = np.concatenate([res.results[c]["out"] for c in range(N_CORES)], axis=0)
    return out.reshape(B, S, N), res


def kernel(x, weight, scale_weight, bias):
    out, _ = _run({"x": x, "weight": weight, "scale_weight": scale_weight, "bias": bias})
    return out
